# revision 6
# baseline (speedup 1.0000x reference)
"""Distributed ARMAConv kernel for 8 TRN2 NeuronCores (Bass/Tile).

Reference computation (N=16384 nodes, F=64 in-feats, C=32 channels,
K=2 stacks, T=2 iterations):
    for each stack k:  xbar = x
        for i in 0..1: xbar = relu(fltr @ (xbar @ w1) + x @ w2 + b)
    out = mean over stacks                                  -> [N, 32]

Strategy:
  - Row-shard fltr across 8 cores; core m holds fltr[rows_m, :] stored
    TRANSPOSED ([N, 2048], contraction-major) so TensorE tiles load as
    contiguous DMA.
  - Fuse the two independent stacks: Y = [xbar_k0 @ w1_k0 | xbar_k1 @ w1_k1]
    is [N, 64], so fltr is streamed from HBM only twice (once per
    iteration) instead of four times.  This is the memory roofline.
  - Iteration 0 needs no communication (x is replicated).  Between the
    iterations, one all-gather of Y1 = xbar1 @ w1 ([N, 64]), split into
    8 row-chunks so pass 2's compute overlaps the collective.
  - fltr is read from HBM at full f32 width but cast to bf16 inside the
    DMA datapath (gpsimd SWDGE cast) so the TensorEngine runs at
    1 cyc/row; fp32r left the PE as the bottleneck (~2.7 cyc/row).
  - All big matmuls run transposed (out^T = Y^T @ fltr_m^T) so the moving
    operand streams 512 rows/instr.
  - relu positive homogeneity folds the final stack-mean 0.5 scale into
    the pass-2 activation.
"""

import numpy as np

import concourse.mybir as mybir
import concourse.tile as tile
from concourse import bacc
from concourse.bass_utils import run_bass_kernel_spmd

N = 16384            # nodes
F = 64               # input features
C = 32               # channels per stack
C2 = 2 * C           # fused channels (2 stacks)
NCORES = 8
R = N // NCORES      # fltr rows per core (2048)
P = 128              # partitions
NKT = N // P         # K tiles per full pass (128)
RC = 4               # output row chunks per core
RCW = R // RC        # 512
GC = 8               # gather chunks
GW = R // GC         # 256 local rows per gather chunk
XCHUNK = 1024        # xT DMA chunk width
KB1 = 4              # K tiles per pass-1 fltr DMA (4 MiB reads)
KB2 = 2              # K tiles per pass-2 fltr DMA (2 MiB reads)

F32 = mybir.dt.float32
F32R = mybir.dt.float32r
BF16 = mybir.dt.bfloat16

_CACHE = {}


def _build():
    nc = bacc.Bacc(
        trn_type="TRN2", target_bir_lowering=False, debug=False,
        num_devices=NCORES,
    )
    fltrT_e = nc.dram_tensor("fltrt", [N, R], F32, kind="ExternalInput")
    xT_e = nc.dram_tensor("xt", [F, N], F32, kind="ExternalInput")
    xtm_e = nc.dram_tensor("xtm", [F, R], F32, kind="ExternalInput")
    w1i0_e = nc.dram_tensor("w1i0", [F, C2], F32, kind="ExternalInput")
    w1i1_e = nc.dram_tensor("w1i1", [C2, C2], F32, kind="ExternalInput")
    w2i0_e = nc.dram_tensor("w2i0", [F, C2], F32, kind="ExternalInput")
    w2i1_e = nc.dram_tensor("w2i1", [F, C2], F32, kind="ExternalInput")
    bi0_e = nc.dram_tensor("bi0", [C2, 1], F32, kind="ExternalInput")
    bi1h_e = nc.dram_tensor("bi1h", [C2, 1], F32, kind="ExternalInput")
    out_e = nc.dram_tensor("out", [C, R], F32, kind="ExternalOutput")

    RG = [list(range(NCORES))]

    with tile.TileContext(nc) as tc:
        with (
            tc.tile_pool(name="wpool", bufs=1) as wpool,
            tc.tile_pool(name="xcpool", bufs=3) as xcpool,
            tc.tile_pool(name="y0pool", bufs=1) as y0pool,
            tc.tile_pool(name="fpool", bufs=4) as fpool,
            tc.tile_pool(name="xbpool", bufs=4) as xbpool,
            tc.tile_pool(name="ylpool", bufs=4) as ylpool,
            tc.tile_pool(name="ygpool", bufs=3) as ygpool,
            tc.tile_pool(name="opool", bufs=1) as opool,
            tc.tile_pool(name="pacc", bufs=4, space="PSUM") as pacc,
            tc.tile_pool(name="psmall", bufs=2, space="PSUM") as psmall,
            tc.tile_pool(name="dram", bufs=8, space="DRAM") as dram,
        ):
            # resident small tensors
            w1i0 = wpool.tile([F, C2], F32)
            nc.sync.dma_start(w1i0[:], w1i0_e[:])
            w1i1 = wpool.tile([C2, C2], F32)  # block-diag [w1_k0i1, w1_k1i1]
            nc.sync.dma_start(w1i1[:], w1i1_e[:])
            w2i0 = wpool.tile([F, C2], F32R)
            nc.sync.dma_start(w2i0[:], w2i0_e[:].bitcast(F32R))
            w2i1 = wpool.tile([F, C2], F32R)
            nc.sync.dma_start(w2i1[:], w2i1_e[:].bitcast(F32R))
            bi0 = wpool.tile([C2, 1], F32)
            nc.sync.dma_start(bi0[:], bi0_e[:])
            bi1h = wpool.tile([C2, 1], F32)
            nc.sync.dma_start(bi1h[:], bi1h_e[:])
            xm = wpool.tile([F, R], F32R)
            nc.sync.dma_start(xm[:], xtm_e[:].bitcast(F32R))

            y0 = y0pool.tile([P, NKT, C2], BF16)  # node-major Y0 (lhsT tiles)

            # ---- Y0 = x @ [w1_k0i0 | w1_k1i0], node-major, cast to bf16 ----
            for g in range(N // XCHUNK):  # 16 groups of 8 kt
                xc = xcpool.tile([F, XCHUNK], F32, name="xc")
                nc.sync.dma_start(xc[:], xT_e[:, g * XCHUNK:(g + 1) * XCHUNK])
                ps0 = psmall.tile([P, 8, C2], F32, name="ps0", tag="ps0")
                for i in range(8):
                    nc.tensor.matmul(
                        ps0[:, i, :],
                        xc[:, i * P:(i + 1) * P],
                        w1i0[:],
                        start=True, stop=True,
                    )
                nc.vector.tensor_copy(y0[:, g * 8:(g + 1) * 8, :], ps0[:])

            # ---- pass 1 in two row-halves so the first half's all-gathers
            # ---- fire at mid-pass and hide completely behind the stream
            gouts = []
            HW_ = R // 2  # 1024 rows per half
            for half in range(2):
                p1 = []
                for rc2 in range(2):
                    rc = half * 2 + rc2
                    acc = pacc.tile([C2, RCW], F32, name=f"p1_{rc}", tag="acc")
                    nc.tensor.matmul(
                        acc[:],
                        w2i0[:],
                        xm[:, rc * RCW:(rc + 1) * RCW],
                        start=True, stop=False,
                    )
                    p1.append(acc)

                for ktb in range(NKT // KB1):
                    ft = fpool.tile([P, KB1, HW_], BF16, name="ft", tag="ft",
                                    bufs=4)
                    nc.gpsimd.dma_start(
                        ft[:],
                        fltrT_e[ktb * KB1 * P:(ktb + 1) * KB1 * P,
                                half * HW_:(half + 1) * HW_]
                        .rearrange("(b p) c -> p b c", p=P),
                    )
                    for b in range(KB1):
                        kt = ktb * KB1 + b
                        for rc2 in range(2):
                            nc.tensor.matmul(
                                p1[rc2][:],
                                y0[:, kt, :],
                                ft[:, b, rc2 * RCW:(rc2 + 1) * RCW],
                                start=False, stop=(kt == NKT - 1),
                            )

                for rc2 in range(2):
                    rc = half * 2 + rc2
                    xb1 = xbpool.tile([C2, RCW], F32, name="xb1")
                    nc.scalar.activation(
                        xb1[:], p1[rc2][:], mybir.ActivationFunctionType.Relu,
                        bias=bi0[:], scale=1.0,
                    )
                    y1l = ylpool.tile([P, RC, C2], BF16, name="y1l")
                    for t in range(RC):  # node-subtiles of 128 in the chunk
                        psy = psmall.tile([P, C2], F32, name="psy", tag="psy")
                        nc.tensor.matmul(
                            psy[:],
                            xb1[:, t * P:(t + 1) * P],
                            w1i1[:],
                            start=True, stop=True,
                        )
                        nc.vector.tensor_copy(y1l[:, t, :], psy[:])
                    for h in range(2):
                        gin = dram.tile([GW, C2], BF16, name="gin", tag="gin")
                        nc.sync.dma_start(
                            gin[:].rearrange("(t p) ch -> p t ch", p=P),
                            y1l[:, 2 * h:2 * h + 2, :],
                        )
                        gout = dram.tile(
                            [NCORES * GW, C2], BF16, name="gout", tag="gout",
                            addr_space="Shared",
                        )
                        nc.gpsimd.collective_compute(
                            "AllGather", mybir.AluOpType.bypass,
                            replica_groups=RG,
                            ins=[gin[:].opt()], outs=[gout[:].opt()],
                        )
                        gouts.append(gout)

            # ---- pass 2 accumulators ----
            p2 = []
            for rc in range(RC):
                acc = pacc.tile([C2, RCW], F32, name=f"p2_{rc}", tag="acc")
                nc.tensor.matmul(
                    acc[:],
                    w2i1[:],
                    xm[:, rc * RCW:(rc + 1) * RCW],
                    start=True, stop=False,
                )
                p2.append(acc)

            # ---- pass 2 main: consume gather chunks in order ----
            for c in range(GC):
                yg = ygpool.tile([P, NCORES * KB2, C2], BF16, name="yg")
                nc.sync.dma_start(
                    yg[:], gouts[c][:].rearrange("(b p) ch -> p b ch", p=P)
                )
                for j in range(NCORES):
                    base = j * R + c * GW
                    ft = fpool.tile([P, KB2, R], BF16, name="ft2", tag="ft2", bufs=6)
                    nc.gpsimd.dma_start(
                        ft[:],
                        fltrT_e[base:base + KB2 * P, :]
                        .rearrange("(b p) c -> p b c", p=P),
                    )
                    for t in range(KB2):
                        last = (c == GC - 1) and (j == NCORES - 1) and (t == KB2 - 1)
                        for rc in range(RC):
                            nc.tensor.matmul(
                                p2[rc][:],
                                yg[:, j * KB2 + t, :],
                                ft[:, t, rc * RCW:(rc + 1) * RCW],
                                start=False, stop=last,
                            )

            # ---- pass 2 epilogue: relu(0.5 z + 0.5 b), stack mean ----
            outT = opool.tile([C, R], F32)
            for rc in range(RC):
                xb2 = xbpool.tile([C2, RCW], F32, name="xb2")
                nc.scalar.activation(
                    xb2[:], p2[rc][:], mybir.ActivationFunctionType.Relu,
                    bias=bi1h[:], scale=0.5,
                )
                # partition-shift stack-1 half to base 0 (DMA), then add
                xs = xbpool.tile([C, RCW], F32, name="xs")
                nc.sync.dma_start(xs[:], xb2[C:C2, :])
                nc.vector.tensor_add(
                    outT[:, rc * RCW:(rc + 1) * RCW],
                    xb2[0:C, :], xs[:],
                )
            nc.sync.dma_start(out_e[:], outT[:])

    nc.compile()
    return nc


def kernel(**inputs):
    x = np.ascontiguousarray(np.asarray(inputs["x"], dtype=np.float32))
    fltr = np.ascontiguousarray(np.asarray(inputs["fltr"], dtype=np.float32))

    def cat(a, b, axis=1):
        return np.ascontiguousarray(
            np.concatenate(
                [np.asarray(a, np.float32), np.asarray(b, np.float32)],
                axis=axis,
            )
        )

    w1i0 = cat(inputs["k0i0_w1"], inputs["k1i0_w1"])
    w1i1 = np.zeros((C2, C2), dtype=np.float32)
    w1i1[0:C, 0:C] = np.asarray(inputs["k0i1_w1"], np.float32)
    w1i1[C:C2, C:C2] = np.asarray(inputs["k1i1_w1"], np.float32)
    w2i0 = cat(inputs["k0i0_w2"], inputs["k1i0_w2"])
    w2i1 = cat(inputs["k0i1_w2"], inputs["k1i1_w2"])
    bi0 = cat(inputs["k0i0_b"], inputs["k1i0_b"], axis=0)[:, None]
    bi1h = 0.5 * cat(inputs["k0i1_b"], inputs["k1i1_b"], axis=0)[:, None]
    bi1h = np.ascontiguousarray(bi1h)
    xT = np.ascontiguousarray(x.T)

    if "nc" not in _CACHE:
        _CACHE["nc"] = _build()
    nc = _CACHE["nc"]

    in_maps = []
    for m in range(NCORES):
        rows = slice(m * R, (m + 1) * R)
        in_maps.append({
            "fltrt": np.ascontiguousarray(fltr[rows, :].T),
            "xt": xT,
            "xtm": np.ascontiguousarray(x[rows, :].T),
            "w1i0": w1i0, "w1i1": w1i1, "w2i0": w2i0, "w2i1": w2i1,
            "bi0": bi0, "bi1h": bi1h,
        })

    import os
    trace = os.environ.get("ARMA_TRACE") == "1"
    res = run_bass_kernel_spmd(
        nc, in_maps, core_ids=list(range(NCORES)), trace=trace,
    )
    _CACHE["last_results"] = res
    out = np.concatenate(
        [np.asarray(res.results[m]["out"]).T for m in range(NCORES)], axis=0
    )
    return out


# revision 11
# speedup vs baseline: 1.2312x; 1.2312x over previous
"""Distributed ARMAConv kernel for 8 TRN2 NeuronCores (Bass/Tile).

Reference computation (N=16384 nodes, F=64 in-feats, C=32 channels,
K=2 stacks, T=2 iterations):
    for each stack k:  xbar = x
        for i in 0..1: xbar = relu(fltr @ (xbar @ w1) + x @ w2 + b)
    out = mean over stacks                                  -> [N, 32]

Strategy:
  - Row-shard fltr across 8 cores; core m holds fltr[rows_m, :] stored
    TRANSPOSED ([N, 2048], contraction-major) so TensorE tiles load as
    contiguous DMA.
  - Fuse the two independent stacks: Y = [xbar_k0 @ w1_k0 | xbar_k1 @ w1_k1]
    is [N, 64], so fltr is streamed from HBM only twice (once per
    iteration) instead of four times.  This is the memory roofline.
  - Iteration 0 needs no communication (x is replicated).  Between the
    iterations, one all-gather of Y1 = xbar1 @ w1 ([N, 64]), split into
    8 row-chunks so pass 2's compute overlaps the collective.
  - fltr is read from HBM at full f32 width but cast to bf16 inside the
    DMA datapath (gpsimd SWDGE cast) so the TensorEngine runs at
    1 cyc/row; fp32r left the PE as the bottleneck (~2.7 cyc/row).
  - All big matmuls run transposed (out^T = Y^T @ fltr_m^T) so the moving
    operand streams 512 rows/instr.
  - relu positive homogeneity folds the final stack-mean 0.5 scale into
    the pass-2 activation.
"""

import numpy as np

import concourse.mybir as mybir
import concourse.tile as tile
from concourse import bacc
from concourse.bass_utils import run_bass_kernel_spmd

N = 16384            # nodes
F = 64               # input features
C = 32               # channels per stack
C2 = 2 * C           # fused channels (2 stacks)
NCORES = 8
R = N // NCORES      # fltr rows per core (2048)
P = 128              # partitions
NKT = N // P         # K tiles per full pass (128)
RC = 4               # output row chunks per core
RCW = R // RC        # 512
GC = 8               # gather chunks
GW = R // GC         # 256 local rows per gather chunk
XCHUNK = 1024        # xT DMA chunk width
KB1 = 4              # K tiles per pass-1 fltr DMA (4 MiB reads)
KB2 = 2              # K tiles per pass-2 fltr DMA (2 MiB reads)

F32 = mybir.dt.float32
F32R = mybir.dt.float32r
BF16 = mybir.dt.bfloat16

_CACHE = {}


def _build():
    nc = bacc.Bacc(
        trn_type="TRN2", target_bir_lowering=False, debug=False,
        num_devices=NCORES,
    )
    fltrT_e = nc.dram_tensor("fltrt", [N, R], F32, kind="ExternalInput")
    xT_e = nc.dram_tensor("xt", [F, N], F32, kind="ExternalInput")
    xtm_e = nc.dram_tensor("xtm", [F, R], F32, kind="ExternalInput")
    w1i0_e = nc.dram_tensor("w1i0", [F, C2], F32, kind="ExternalInput")
    w1i1_e = nc.dram_tensor("w1i1", [C2, C2], F32, kind="ExternalInput")
    w2i0_e = nc.dram_tensor("w2i0", [F, C2], F32, kind="ExternalInput")
    w2i1_e = nc.dram_tensor("w2i1", [F, C2], F32, kind="ExternalInput")
    bi0_e = nc.dram_tensor("bi0", [C2, 1], F32, kind="ExternalInput")
    bi1h_e = nc.dram_tensor("bi1h", [C2, 1], F32, kind="ExternalInput")
    out_e = nc.dram_tensor("out", [C, R], F32, kind="ExternalOutput")

    RG = [list(range(NCORES))]

    with tile.TileContext(nc) as tc:
        with (
            tc.tile_pool(name="wpool", bufs=1) as wpool,
            tc.tile_pool(name="xcpool", bufs=2) as xcpool,
            tc.tile_pool(name="y0pool", bufs=1) as y0pool,
            tc.tile_pool(name="fpool", bufs=4) as fpool,
            tc.tile_pool(name="xbpool", bufs=2) as xbpool,
            tc.tile_pool(name="ylpool", bufs=2) as ylpool,
            tc.tile_pool(name="ygpool", bufs=3) as ygpool,
            tc.tile_pool(name="opool", bufs=1) as opool,
            tc.tile_pool(name="pacc", bufs=4, space="PSUM") as pacc,
            tc.tile_pool(name="psmall", bufs=2, space="PSUM") as psmall,
            tc.tile_pool(name="dram", bufs=8, space="DRAM") as dram,
        ):
            # resident small tensors
            w1i0 = wpool.tile([F, C2], F32)
            nc.sync.dma_start(w1i0[:], w1i0_e[:])
            w1i1 = wpool.tile([C2, C2], F32)  # block-diag [w1_k0i1, w1_k1i1]
            nc.sync.dma_start(w1i1[:], w1i1_e[:])
            w2i0 = wpool.tile([F, C2], F32R)
            nc.sync.dma_start(w2i0[:], w2i0_e[:].bitcast(F32R))
            w2i1 = wpool.tile([F, C2], F32R)
            nc.sync.dma_start(w2i1[:], w2i1_e[:].bitcast(F32R))
            bi0 = wpool.tile([C2, 1], F32)
            nc.sync.dma_start(bi0[:], bi0_e[:])
            bi1h = wpool.tile([C2, 1], F32)
            nc.sync.dma_start(bi1h[:], bi1h_e[:])
            xm = wpool.tile([F, R], F32R)
            nc.sync.dma_start(xm[:], xtm_e[:].bitcast(F32R))

            y0 = y0pool.tile([P, NKT, C2], BF16)  # node-major Y0 (lhsT tiles)

            # ---- Y0 = x @ [w1_k0i0 | w1_k1i0], node-major, cast to bf16 ----
            for g in range(N // XCHUNK):  # 16 groups of 8 kt
                xc = xcpool.tile([F, XCHUNK], F32, name="xc")
                nc.sync.dma_start(xc[:], xT_e[:, g * XCHUNK:(g + 1) * XCHUNK])
                ps0 = psmall.tile([P, 8, C2], F32, name="ps0", tag="ps0")
                for i in range(8):
                    nc.tensor.matmul(
                        ps0[:, i, :],
                        xc[:, i * P:(i + 1) * P],
                        w1i0[:],
                        start=True, stop=True,
                    )
                nc.vector.tensor_copy(y0[:, g * 8:(g + 1) * 8, :], ps0[:])

            # ---- pass 1 in two row-halves so the first half's all-gathers
            # ---- fire at mid-pass and hide completely behind the stream
            gouts = []
            HW_ = R // 2  # 1024 rows per half
            for half in range(2):
                p1 = []
                for rc2 in range(2):
                    rc = half * 2 + rc2
                    acc = pacc.tile([C2, RCW], F32, name=f"p1_{rc}", tag="acc")
                    nc.tensor.matmul(
                        acc[:],
                        w2i0[:],
                        xm[:, rc * RCW:(rc + 1) * RCW],
                        start=True, stop=False,
                    )
                    p1.append(acc)

                for ktb in range(NKT // KB1):
                    ft = fpool.tile([P, KB1, HW_], BF16, name="ft", tag="ft",
                                    bufs=4)
                    nc.gpsimd.dma_start(
                        ft[:],
                        fltrT_e[ktb * KB1 * P:(ktb + 1) * KB1 * P,
                                half * HW_:(half + 1) * HW_]
                        .rearrange("(b p) c -> p b c", p=P),
                    )
                    for b in range(KB1):
                        kt = ktb * KB1 + b
                        for rc2 in range(2):
                            nc.tensor.matmul(
                                p1[rc2][:],
                                y0[:, kt, :],
                                ft[:, b, rc2 * RCW:(rc2 + 1) * RCW],
                                start=False, stop=(kt == NKT - 1),
                            )

                for rc2 in range(2):
                    rc = half * 2 + rc2
                    xb1 = xbpool.tile([C2, RCW], F32, name="xb1")
                    nc.scalar.activation(
                        xb1[:], p1[rc2][:], mybir.ActivationFunctionType.Relu,
                        bias=bi0[:], scale=1.0,
                    )
                    y1l = ylpool.tile([P, RC, C2], BF16, name="y1l")
                    for t in range(RC):  # node-subtiles of 128 in the chunk
                        psy = psmall.tile([P, C2], F32, name="psy", tag="psy")
                        nc.tensor.matmul(
                            psy[:],
                            xb1[:, t * P:(t + 1) * P],
                            w1i1[:],
                            start=True, stop=True,
                        )
                        nc.vector.tensor_copy(y1l[:, t, :], psy[:])
                    for h in range(2):
                        gin = dram.tile([GW, C2], BF16, name="gin", tag="gin")
                        nc.sync.dma_start(
                            gin[:].rearrange("(t p) ch -> p t ch", p=P),
                            y1l[:, 2 * h:2 * h + 2, :],
                        )
                        gout = dram.tile(
                            [NCORES * GW, C2], BF16, name="gout", tag="gout",
                            addr_space="Shared",
                        )
                        nc.gpsimd.collective_compute(
                            "AllGather", mybir.AluOpType.bypass,
                            replica_groups=RG,
                            ins=[gin[:].opt()], outs=[gout[:].opt()],
                        )
                        gouts.append(gout)

            outT = opool.tile([C, R], F32)

            # ---- pass 2: two output-column halves; the first half's
            # ---- epilogue hides under the second half's stream ----
            ygs = []
            for h in range(2):
                yg = ygpool.tile([P, NCORES * NB2G, C2], BF16, name="yg",
                                 bufs=2)
                nc.sync.dma_start(
                    yg[:], gouts[h][:].rearrange("(b p) ch -> p b ch", p=P)
                )
                ygs.append(yg)

            for oh in range(2):
                p2 = []
                for rc2 in range(2):
                    rc = oh * 2 + rc2
                    acc = pacc.tile([C2, RCW], F32, name=f"p2_{rc}", tag="acc")
                    nc.tensor.matmul(
                        acc[:],
                        w2i1[:],
                        xm[:, rc * RCW:(rc + 1) * RCW],
                        start=True, stop=False,
                    )
                    p2.append(acc)
                for h in range(2):
                    for j in range(NCORES):
                        for q in range(2):
                            if oh == 0 and h == 0 and j < 2:
                                ft = pf2_tiles[j * 2 + q]
                            else:
                                ft = ft2_dma(h, j, q, oh)
                            for t in range(4):
                                kt_in = q * 4 + t
                                last = (h == 1) and (j == NCORES - 1) \
                                    and (kt_in == NB2G - 1)
                                for rc2 in range(2):
                                    nc.tensor.matmul(
                                        p2[rc2][:],
                                        ygs[h][:, j * NB2G + kt_in, :],
                                        ft[:, t, rc2 * RCW:(rc2 + 1) * RCW],
                                        start=False, stop=last,
                                    )
                # epilogue for this output half
                for rc2 in range(2):
                    rc = oh * 2 + rc2
                    xb2 = xbpool.tile([C2, RCW], F32, name="xb2")
                    nc.scalar.activation(
                        xb2[:], p2[rc2][:], mybir.ActivationFunctionType.Relu,
                        bias=bi1h[:], scale=0.5,
                    )
                    # partition-shift stack-1 half to base 0 (DMA), then add
                    xs = xbpool.tile([C, RCW], F32, name="xs")
                    nc.sync.dma_start(xs[:], xb2[C:C2, :])
                    nc.vector.tensor_add(
                        outT[:, rc * RCW:(rc + 1) * RCW],
                        xb2[0:C, :], xs[:],
                    )
            nc.sync.dma_start(out_e[:], outT[:])

    nc.compile()
    return nc


def kernel(**inputs):
    x = np.ascontiguousarray(np.asarray(inputs["x"], dtype=np.float32))
    fltr = np.ascontiguousarray(np.asarray(inputs["fltr"], dtype=np.float32))

    def cat(a, b, axis=1):
        return np.ascontiguousarray(
            np.concatenate(
                [np.asarray(a, np.float32), np.asarray(b, np.float32)],
                axis=axis,
            )
        )

    w1i0 = cat(inputs["k0i0_w1"], inputs["k1i0_w1"])
    w1i1 = np.zeros((C2, C2), dtype=np.float32)
    w1i1[0:C, 0:C] = np.asarray(inputs["k0i1_w1"], np.float32)
    w1i1[C:C2, C:C2] = np.asarray(inputs["k1i1_w1"], np.float32)
    w2i0 = cat(inputs["k0i0_w2"], inputs["k1i0_w2"])
    w2i1 = cat(inputs["k0i1_w2"], inputs["k1i1_w2"])
    bi0 = cat(inputs["k0i0_b"], inputs["k1i0_b"], axis=0)[:, None]
    bi1h = 0.5 * cat(inputs["k0i1_b"], inputs["k1i1_b"], axis=0)[:, None]
    bi1h = np.ascontiguousarray(bi1h)
    xT = np.ascontiguousarray(x.T)

    if "nc" not in _CACHE:
        _CACHE["nc"] = _build()
    nc = _CACHE["nc"]

    in_maps = []
    for m in range(NCORES):
        rows = slice(m * R, (m + 1) * R)
        in_maps.append({
            "fltrt": np.ascontiguousarray(fltr[rows, :].T),
            "xt": xT,
            "xtm": np.ascontiguousarray(x[rows, :].T),
            "w1i0": w1i0, "w1i1": w1i1, "w2i0": w2i0, "w2i1": w2i1,
            "bi0": bi0, "bi1h": bi1h,
        })

    import os
    import time
    trace = os.environ.get("ARMA_TRACE") == "1"
    last_exc = None
    for attempt in range(3):
        try:
            res = run_bass_kernel_spmd(
                nc, in_maps, core_ids=list(range(NCORES)), trace=trace,
            )
            break
        except Exception as e:  # transient NRT device errors: retry
            last_exc = e
            time.sleep(5.0)
    else:
        raise last_exc
    _CACHE["last_results"] = res
    out = np.concatenate(
        [np.asarray(res.results[m]["out"]).T for m in range(NCORES)], axis=0
    )
    return out


# revision 13
# speedup vs baseline: 1.2340x; 1.0023x over previous
"""Distributed ARMAConv kernel for 8 TRN2 NeuronCores (Bass/Tile).

Reference computation (N=16384 nodes, F=64 in-feats, C=32 channels,
K=2 stacks, T=2 iterations):
    for each stack k:  xbar = x
        for i in 0..1: xbar = relu(fltr @ (xbar @ w1) + x @ w2 + b)
    out = mean over stacks                                  -> [N, 32]

Strategy:
  - Row-shard fltr across 8 cores; core m holds fltr[rows_m, :] stored
    TRANSPOSED ([N, 2048], contraction-major) so TensorE tiles load as
    contiguous DMA.
  - Fuse the two independent stacks: Y = [xbar_k0 @ w1_k0 | xbar_k1 @ w1_k1]
    is [N, 64], so fltr is streamed from HBM only twice (once per
    iteration) instead of four times.  This is the memory roofline.
  - Iteration 0 needs no communication (x is replicated).  Between the
    iterations, one all-gather of Y1 = xbar1 @ w1 ([N, 64]), split into
    8 row-chunks so pass 2's compute overlaps the collective.
  - fltr is read from HBM at full f32 width but cast to bf16 inside the
    DMA datapath (gpsimd SWDGE cast) so the TensorEngine runs at
    1 cyc/row; fp32r left the PE as the bottleneck (~2.7 cyc/row).
  - All big matmuls run transposed (out^T = Y^T @ fltr_m^T) so the moving
    operand streams 512 rows/instr.
  - relu positive homogeneity folds the final stack-mean 0.5 scale into
    the pass-2 activation.
"""

import numpy as np

import concourse.mybir as mybir
import concourse.tile as tile
from concourse import bacc
from concourse.bass_utils import run_bass_kernel_spmd

N = 16384            # nodes
F = 64               # input features
C = 32               # channels per stack
C2 = 2 * C           # fused channels (2 stacks)
NCORES = 8
R = N // NCORES      # fltr rows per core (2048)
P = 128              # partitions
NKT = N // P         # K tiles per full pass (128)
RC = 4               # output row chunks per core
RCW = R // RC        # 512
GC = 8               # gather chunks
GW = R // GC         # 256 local rows per gather chunk
XCHUNK = 1024        # xT DMA chunk width
KB1 = 4              # K tiles per pass-1 fltr DMA (4 MiB reads)
KB2 = 2              # K tiles per pass-2 fltr DMA (2 MiB reads)

F32 = mybir.dt.float32
F32R = mybir.dt.float32r
BF16 = mybir.dt.bfloat16

_CACHE = {}


def _build():
    nc = bacc.Bacc(
        trn_type="TRN2", target_bir_lowering=False, debug=False,
        num_devices=NCORES,
    )
    fltrT0_e = nc.dram_tensor("fltrt0", [N, R // 2], F32, kind="ExternalInput")
    fltrT1_e = nc.dram_tensor("fltrt1", [N, R // 2], F32, kind="ExternalInput")
    xT_e = nc.dram_tensor("xt", [F, N], F32, kind="ExternalInput")
    xtm_e = nc.dram_tensor("xtm", [F, R], F32, kind="ExternalInput")
    w1i0_e = nc.dram_tensor("w1i0", [F, C2], F32, kind="ExternalInput")
    w1i1_e = nc.dram_tensor("w1i1", [C2, C2], F32, kind="ExternalInput")
    w2i0_e = nc.dram_tensor("w2i0", [F, C2], F32, kind="ExternalInput")
    w2i1_e = nc.dram_tensor("w2i1", [F, C2], F32, kind="ExternalInput")
    bi0_e = nc.dram_tensor("bi0", [C2, 1], F32, kind="ExternalInput")
    bi1h_e = nc.dram_tensor("bi1h", [C2, 1], F32, kind="ExternalInput")
    out_e = nc.dram_tensor("out", [C, R], F32, kind="ExternalOutput")

    RG = [list(range(NCORES))]

    with tile.TileContext(nc) as tc:
        with (
            tc.tile_pool(name="wpool", bufs=1) as wpool,
            tc.tile_pool(name="xcpool", bufs=2) as xcpool,
            tc.tile_pool(name="y0pool", bufs=1) as y0pool,
            tc.tile_pool(name="fpool", bufs=4) as fpool,
            tc.tile_pool(name="xbpool", bufs=2) as xbpool,
            tc.tile_pool(name="ylpool", bufs=2) as ylpool,
            tc.tile_pool(name="ygpool", bufs=3) as ygpool,
            tc.tile_pool(name="opool", bufs=1) as opool,
            tc.tile_pool(name="pacc", bufs=4, space="PSUM") as pacc,
            tc.tile_pool(name="psmall", bufs=2, space="PSUM") as psmall,
            tc.tile_pool(name="dram", bufs=8, space="DRAM") as dram,
        ):
            # resident small tensors
            w1i0 = wpool.tile([F, C2], F32)
            nc.sync.dma_start(w1i0[:], w1i0_e[:])
            w1i1 = wpool.tile([C2, C2], F32)  # block-diag [w1_k0i1, w1_k1i1]
            nc.sync.dma_start(w1i1[:], w1i1_e[:])
            w2i0 = wpool.tile([F, C2], F32R)
            nc.sync.dma_start(w2i0[:], w2i0_e[:].bitcast(F32R))
            w2i1 = wpool.tile([F, C2], F32R)
            nc.sync.dma_start(w2i1[:], w2i1_e[:].bitcast(F32R))
            bi0 = wpool.tile([C2, 1], F32)
            nc.sync.dma_start(bi0[:], bi0_e[:])
            bi1h = wpool.tile([C2, 1], F32)
            nc.sync.dma_start(bi1h[:], bi1h_e[:])
            xm = wpool.tile([F, R], F32R)
            nc.sync.dma_start(xm[:], xtm_e[:].bitcast(F32R))

            y0 = y0pool.tile([P, NKT, C2], BF16)  # node-major Y0 (lhsT tiles)

            # ---- Y0 = x @ [w1_k0i0 | w1_k1i0], node-major, cast to bf16 ----
            for g in range(N // XCHUNK):  # 16 groups of 8 kt
                xc = xcpool.tile([F, XCHUNK], F32, name="xc")
                nc.sync.dma_start(xc[:], xT_e[:, g * XCHUNK:(g + 1) * XCHUNK])
                ps0 = psmall.tile([P, 8, C2], F32, name="ps0", tag="ps0")
                for i in range(8):
                    nc.tensor.matmul(
                        ps0[:, i, :],
                        xc[:, i * P:(i + 1) * P],
                        w1i0[:],
                        start=True, stop=True,
                    )
                nc.vector.tensor_copy(y0[:, g * 8:(g + 1) * 8, :], ps0[:])

            # ---- pass 1 in two row-halves: each half's single all-gather
            # ---- fires at mid-stream; the next phase's fltr DMAs are
            # ---- prefetched ahead of the collective on the gpsimd queue
            HW_ = R // 2          # 1024 output rows per half
            NB2G = HW_ // P       # 8 K-tiles per (half, core) gather block
            NKB1 = NKT // KB1     # 32 fltr DMAs per half
            PF = 6                # half-1 tiles prefetched before gather 0
            gouts = []
            pf_tiles = []

            def p1_conv(p1, ft, ktb):
                for b in range(KB1):
                    kt = ktb * KB1 + b
                    for rc2 in range(2):
                        nc.tensor.matmul(
                            p1[rc2][:],
                            y0[:, kt, :],
                            ft[:, b, rc2 * RCW:(rc2 + 1) * RCW],
                            start=False, stop=(kt == NKT - 1),
                        )

            fltr_halves = [fltrT0_e, fltrT1_e]

            def ft_dma(half, ktb):
                ft = fpool.tile([P, KB1, HW_], BF16, name="ft", tag="ft",
                                bufs=4)
                nc.gpsimd.dma_start(
                    ft[:],
                    fltr_halves[half][ktb * KB1 * P:(ktb + 1) * KB1 * P, :]
                    .rearrange("(b p) c -> p b c", p=P),
                )
                return ft

            def ft2_dma(h, j, q, oh):
                ft = fpool.tile([P, 4, HW_], BF16, name="ft2",
                                tag="ft2", bufs=7)
                base = j * R + h * HW_ + q * (HW_ // 2)
                nc.gpsimd.dma_start(
                    ft[:],
                    fltr_halves[oh][base:base + HW_ // 2, :]
                    .rearrange("(b p) c -> p b c", p=P),
                )
                return ft

            for half in range(2):
                p1 = []
                for rc2 in range(2):
                    rc = half * 2 + rc2
                    acc = pacc.tile([C2, RCW], F32, name=f"p1_{rc}", tag="acc")
                    nc.tensor.matmul(
                        acc[:],
                        w2i0[:],
                        xm[:, rc * RCW:(rc + 1) * RCW],
                        start=True, stop=False,
                    )
                    p1.append(acc)

                kept = {}
                for ktb in range(NKB1):
                    if half == 1 and ktb < PF:
                        ft = pf_tiles[ktb]
                    elif half == 1 and ktb >= NKB1 - 4:
                        # pin the tiles pass 2 needs for its (oh=1, j=7)
                        # group so they are not re-read from HBM
                        ft = fpool.tile([P, KB1, HW_], BF16, name="ftk",
                                        tag="ftk", bufs=4)
                        nc.gpsimd.dma_start(
                            ft[:],
                            fltrT1_e[ktb * KB1 * P:(ktb + 1) * KB1 * P, :]
                            .rearrange("(b p) c -> p b c", p=P),
                        )
                        kept[ktb] = ft
                    else:
                        ft = ft_dma(half, ktb)
                    p1_conv(p1, ft, ktb)
                if half == 1:
                    kept_tiles = kept

                if half == 0:
                    # prefetch half-1's first tiles so the SDMA engines stay
                    # fed while the collective blocks the gpsimd queue
                    pf_tiles = [ft_dma(1, k) for k in range(PF)]
                else:
                    # prefetch pass-2's first tiles for the same reason
                    pf2_tiles = [ft2_dma(0, 0, 0, 0), ft2_dma(0, 0, 1, 0),
                                 ft2_dma(0, 1, 0, 0), ft2_dma(0, 1, 1, 0)]

                # epilogue: relu -> Y1 local (bf16) -> one all-gather
                y1h = ylpool.tile([P, 8, C2], BF16, name="y1h")
                for rc2 in range(2):
                    rc = half * 2 + rc2
                    xb1 = xbpool.tile([C2, RCW], F32, name="xb1")
                    nc.scalar.activation(
                        xb1[:], p1[rc2][:], mybir.ActivationFunctionType.Relu,
                        bias=bi0[:], scale=1.0,
                    )
                    for t in range(RC):  # node-subtiles of 128 in the chunk
                        psy = psmall.tile([P, C2], F32, name="psy", tag="psy")
                        nc.tensor.matmul(
                            psy[:],
                            xb1[:, t * P:(t + 1) * P],
                            w1i1[:],
                            start=True, stop=True,
                        )
                        nc.vector.tensor_copy(y1h[:, rc2 * RC + t, :], psy[:])
                gin = dram.tile([HW_, C2], BF16, name="gin", tag="gin", bufs=2)
                nc.sync.dma_start(
                    gin[:].rearrange("(t p) ch -> p t ch", p=P),
                    y1h[:],
                )
                gout = dram.tile(
                    [NCORES * HW_, C2], BF16, name="gout", tag="gout",
                    addr_space="Shared", bufs=2,
                )
                nc.gpsimd.collective_compute(
                    "AllGather", mybir.AluOpType.bypass,
                    replica_groups=RG,
                    ins=[gin[:].opt()], outs=[gout[:].opt()],
                )
                gouts.append(gout)

            outT = opool.tile([C, R], F32)

            # ---- pass 2: two output-column halves; the first half's
            # ---- epilogue hides under the second half's stream ----
            ygs = []
            for h in range(2):
                yg = ygpool.tile([P, NCORES * NB2G, C2], BF16, name="yg",
                                 bufs=2)
                nc.sync.dma_start(
                    yg[:], gouts[h][:].rearrange("(b p) ch -> p b ch", p=P)
                )
                ygs.append(yg)

            for oh in range(2):
                p2 = []
                for rc2 in range(2):
                    rc = oh * 2 + rc2
                    acc = pacc.tile([C2, RCW], F32, name=f"p2_{rc}", tag="acc")
                    nc.tensor.matmul(
                        acc[:],
                        w2i1[:],
                        xm[:, rc * RCW:(rc + 1) * RCW],
                        start=True, stop=False,
                    )
                    p2.append(acc)
                jorder = list(range(NCORES)) if oh == 0 \
                    else [NCORES - 1] + list(range(NCORES - 1))
                n_done = 0
                for h in range(2):
                    for j in jorder:
                        for q in range(2):
                            n_done += 1
                            if oh == 0 and h == 0 and j < 2:
                                ft = pf2_tiles[j * 2 + q]
                            elif oh == 1 and j == NCORES - 1:
                                # pinned from pass 1: rows j*2048+h*1024+q*512
                                # = ktb 28 + h*2 + q, cols [1024:2048]
                                ft = kept_tiles[28 + h * 2 + q]
                            else:
                                ft = ft2_dma(h, j, q, oh)
                            for t in range(4):
                                kt_in = q * 4 + t
                                last = (n_done == 2 * NCORES * 2) and (t == 3)
                                for rc2 in range(2):
                                    nc.tensor.matmul(
                                        p2[rc2][:],
                                        ygs[h][:, j * NB2G + kt_in, :],
                                        ft[:, t, rc2 * RCW:(rc2 + 1) * RCW],
                                        start=False, stop=last,
                                    )
                # epilogue for this output half
                for rc2 in range(2):
                    rc = oh * 2 + rc2
                    xb2 = xbpool.tile([C2, RCW], F32, name="xb2")
                    nc.scalar.activation(
                        xb2[:], p2[rc2][:], mybir.ActivationFunctionType.Relu,
                        bias=bi1h[:], scale=0.5,
                    )
                    # partition-shift stack-1 half to base 0 (DMA), then add
                    xs = xbpool.tile([C, RCW], F32, name="xs")
                    nc.sync.dma_start(xs[:], xb2[C:C2, :])
                    nc.vector.tensor_add(
                        outT[:, rc * RCW:(rc + 1) * RCW],
                        xb2[0:C, :], xs[:],
                    )
            nc.sync.dma_start(out_e[:], outT[:])

    nc.compile()
    return nc


def kernel(**inputs):
    x = np.ascontiguousarray(np.asarray(inputs["x"], dtype=np.float32))
    fltr = np.ascontiguousarray(np.asarray(inputs["fltr"], dtype=np.float32))

    def cat(a, b, axis=1):
        return np.ascontiguousarray(
            np.concatenate(
                [np.asarray(a, np.float32), np.asarray(b, np.float32)],
                axis=axis,
            )
        )

    w1i0 = cat(inputs["k0i0_w1"], inputs["k1i0_w1"])
    w1i1 = np.zeros((C2, C2), dtype=np.float32)
    w1i1[0:C, 0:C] = np.asarray(inputs["k0i1_w1"], np.float32)
    w1i1[C:C2, C:C2] = np.asarray(inputs["k1i1_w1"], np.float32)
    w2i0 = cat(inputs["k0i0_w2"], inputs["k1i0_w2"])
    w2i1 = cat(inputs["k0i1_w2"], inputs["k1i1_w2"])
    bi0 = cat(inputs["k0i0_b"], inputs["k1i0_b"], axis=0)[:, None]
    bi1h = 0.5 * cat(inputs["k0i1_b"], inputs["k1i1_b"], axis=0)[:, None]
    bi1h = np.ascontiguousarray(bi1h)
    xT = np.ascontiguousarray(x.T)

    if "nc" not in _CACHE:
        _CACHE["nc"] = _build()
    nc = _CACHE["nc"]

    in_maps = []
    for m in range(NCORES):
        rows = slice(m * R, (m + 1) * R)
        in_maps.append({
            "fltrt0": np.ascontiguousarray(fltr[m * R:m * R + R // 2, :].T),
            "fltrt1": np.ascontiguousarray(fltr[m * R + R // 2:(m + 1) * R, :].T),
            "xt": xT,
            "xtm": np.ascontiguousarray(x[rows, :].T),
            "w1i0": w1i0, "w1i1": w1i1, "w2i0": w2i0, "w2i1": w2i1,
            "bi0": bi0, "bi1h": bi1h,
        })

    import os
    import time
    trace = os.environ.get("ARMA_TRACE") == "1"
    last_exc = None
    for attempt in range(3):
        try:
            res = run_bass_kernel_spmd(
                nc, in_maps, core_ids=list(range(NCORES)), trace=trace,
            )
            break
        except Exception as e:  # transient NRT device errors: retry
            last_exc = e
            time.sleep(5.0)
    else:
        raise last_exc
    _CACHE["last_results"] = res
    out = np.concatenate(
        [np.asarray(res.results[m]["out"]).T for m in range(NCORES)], axis=0
    )
    return out



# revision 14
# speedup vs baseline: 1.2460x; 1.0098x over previous
"""Distributed ARMAConv kernel for 8 TRN2 NeuronCores (Bass/Tile).

Reference computation (N=16384 nodes, F=64 in-feats, C=32 channels,
K=2 stacks, T=2 iterations):
    for each stack k:  xbar = x
        for i in 0..1: xbar = relu(fltr @ (xbar @ w1) + x @ w2 + b)
    out = mean over stacks                                  -> [N, 32]

Strategy (measured 884 us on HW vs ~800 us per-core HBM roofline):
  - Row-shard fltr across 8 cores; core m holds fltr[rows_m, :] stored
    TRANSPOSED (contraction-major, split into two contiguous half-arrays)
    so every TensorE tile is a large contiguous DMA read.
  - Fuse the two independent ARMA stacks: Y = [xbar_k0 @ w1_k0 |
    xbar_k1 @ w1_k1] is [N, 64], so fltr streams from HBM only twice
    (once per iteration) instead of four times - the memory roofline.
  - Iteration 0 needs no communication (x is replicated).  Between the
    iterations, Y1 = xbar1 @ w1 ([N, 64]) is all-gathered.  Pass 1 runs
    in two output-row halves so the first half's gather fires at
    mid-stream and hides completely; collective_compute blocks the
    gpsimd queue (which also issues the cast-DMAs), so the next phase's
    fltr tiles are prefetched ahead of each collective in queue order.
  - fltr is read from HBM at full f32 width but cast to bf16 inside the
    DMA datapath (gpsimd SWDGE cast-DMA) so the TensorEngine runs at
    1 cyc/row; fp32r keeps full precision for the small skip-term
    matmuls.  bf16 conv + f32 PSUM accumulate gives rel err ~2e-3.
  - All big matmuls run transposed (out^T = Y^T @ fltr_m^T) so the
    moving operand streams 512 rows/instr; Y tiles are the stationary
    operand (weight loads hide under the previous matmul).
  - Pass 2 runs in two output-column halves so the first epilogue hides
    under the second stream; pass 1's last four tiles stay pinned in
    SBUF and pass 2 consumes them first in its second half (8 MiB of
    HBM reads saved and no DMA tail).
  - relu positive homogeneity folds the final stack-mean 0.5 scale into
    the pass-2 activation; the host only shards/transposes inputs and
    concatenates/transposes the [32, 2048] per-core outputs.
"""

import numpy as np

import concourse.mybir as mybir
import concourse.tile as tile
from concourse import bacc
from concourse.bass_utils import run_bass_kernel_spmd

N = 16384            # nodes
F = 64               # input features
C = 32               # channels per stack
C2 = 2 * C           # fused channels (2 stacks)
NCORES = 8
R = N // NCORES      # fltr rows per core (2048)
P = 128              # partitions
NKT = N // P         # K tiles per full pass (128)
RC = 4               # output row chunks per core
RCW = R // RC        # 512
XCHUNK = 1024        # xT DMA chunk width
KB1 = 4              # K tiles per pass-1 fltr DMA (4 MiB f32 reads)

F32 = mybir.dt.float32
F32R = mybir.dt.float32r
BF16 = mybir.dt.bfloat16

_CACHE = {}


def _build():
    nc = bacc.Bacc(
        trn_type="TRN2", target_bir_lowering=False, debug=False,
        num_devices=NCORES,
    )
    fltrT0_e = nc.dram_tensor("fltrt0", [N, R // 2], F32, kind="ExternalInput")
    fltrT1_e = nc.dram_tensor("fltrt1", [N, R // 2], F32, kind="ExternalInput")
    xT_e = nc.dram_tensor("xt", [F, N], F32, kind="ExternalInput")
    xtm_e = nc.dram_tensor("xtm", [F, R], F32, kind="ExternalInput")
    w1i0_e = nc.dram_tensor("w1i0", [F, C2], F32, kind="ExternalInput")
    w1i1_e = nc.dram_tensor("w1i1", [C2, C2], F32, kind="ExternalInput")
    w2i0_e = nc.dram_tensor("w2i0", [F, C2], F32, kind="ExternalInput")
    w2i1_e = nc.dram_tensor("w2i1", [F, C2], F32, kind="ExternalInput")
    bi0_e = nc.dram_tensor("bi0", [C2, 1], F32, kind="ExternalInput")
    bi1h_e = nc.dram_tensor("bi1h", [C2, 1], F32, kind="ExternalInput")
    out_e = nc.dram_tensor("out", [C, R], F32, kind="ExternalOutput")

    RG = [list(range(NCORES))]

    with tile.TileContext(nc) as tc:
        with (
            tc.tile_pool(name="wpool", bufs=1) as wpool,
            tc.tile_pool(name="xcpool", bufs=2) as xcpool,
            tc.tile_pool(name="y0pool", bufs=1) as y0pool,
            tc.tile_pool(name="fpool", bufs=4) as fpool,
            tc.tile_pool(name="xbpool", bufs=2) as xbpool,
            tc.tile_pool(name="ylpool", bufs=2) as ylpool,
            tc.tile_pool(name="ygpool", bufs=3) as ygpool,
            tc.tile_pool(name="opool", bufs=1) as opool,
            tc.tile_pool(name="pacc", bufs=4, space="PSUM") as pacc,
            tc.tile_pool(name="psmall", bufs=2, space="PSUM") as psmall,
            tc.tile_pool(name="dram", bufs=8, space="DRAM") as dram,
        ):
            # resident small tensors
            w1i0 = wpool.tile([F, C2], F32)
            nc.sync.dma_start(w1i0[:], w1i0_e[:])
            w1i1 = wpool.tile([C2, C2], F32)  # block-diag [w1_k0i1, w1_k1i1]
            nc.sync.dma_start(w1i1[:], w1i1_e[:])
            w2i0 = wpool.tile([F, C2], F32R)
            nc.sync.dma_start(w2i0[:], w2i0_e[:].bitcast(F32R))
            w2i1 = wpool.tile([F, C2], F32R)
            nc.sync.dma_start(w2i1[:], w2i1_e[:].bitcast(F32R))
            bi0 = wpool.tile([C2, 1], F32)
            nc.sync.dma_start(bi0[:], bi0_e[:])
            bi1h = wpool.tile([C2, 1], F32)
            nc.sync.dma_start(bi1h[:], bi1h_e[:])
            xm = wpool.tile([F, R], F32R)
            nc.sync.dma_start(xm[:], xtm_e[:].bitcast(F32R))

            y0 = y0pool.tile([P, NKT, C2], BF16)  # node-major Y0 (lhsT tiles)

            # ---- Y0 = x @ [w1_k0i0 | w1_k1i0], node-major, cast to bf16 ----
            for g in range(N // XCHUNK):  # 16 groups of 8 kt
                xc = xcpool.tile([F, XCHUNK], F32, name="xc")
                nc.sync.dma_start(xc[:], xT_e[:, g * XCHUNK:(g + 1) * XCHUNK])
                ps0 = psmall.tile([P, 8, C2], F32, name="ps0", tag="ps0")
                for i in range(8):
                    nc.tensor.matmul(
                        ps0[:, i, :],
                        xc[:, i * P:(i + 1) * P],
                        w1i0[:],
                        start=True, stop=True,
                    )
                nc.vector.tensor_copy(y0[:, g * 8:(g + 1) * 8, :], ps0[:])

            # ---- pass 1 in two row-halves: each half's single all-gather
            # ---- fires at mid-stream; the next phase's fltr DMAs are
            # ---- prefetched ahead of the collective on the gpsimd queue
            HW_ = R // 2          # 1024 output rows per half
            NB2G = HW_ // P       # 8 K-tiles per (half, core) gather block
            NKB1 = NKT // KB1     # 32 fltr DMAs per half
            PF = 6                # half-1 tiles prefetched before gather 0
            gouts = []
            pf_tiles = []

            def p1_conv(p1, ft, ktb):
                for b in range(KB1):
                    kt = ktb * KB1 + b
                    for rc2 in range(2):
                        nc.tensor.matmul(
                            p1[rc2][:],
                            y0[:, kt, :],
                            ft[:, b, rc2 * RCW:(rc2 + 1) * RCW],
                            start=False, stop=(kt == NKT - 1),
                        )

            fltr_halves = [fltrT0_e, fltrT1_e]

            def ft_dma(half, ktb):
                ft = fpool.tile([P, KB1, HW_], BF16, name="ft", tag="ft",
                                bufs=4)
                nc.gpsimd.dma_start(
                    ft[:],
                    fltr_halves[half][ktb * KB1 * P:(ktb + 1) * KB1 * P, :]
                    .rearrange("(b p) c -> p b c", p=P),
                )
                return ft

            def ft2_dma(h, j, q, oh):
                ft = fpool.tile([P, 4, HW_], BF16, name="ft2",
                                tag="ft2", bufs=7)
                base = j * R + h * HW_ + q * (HW_ // 2)
                nc.gpsimd.dma_start(
                    ft[:],
                    fltr_halves[oh][base:base + HW_ // 2, :]
                    .rearrange("(b p) c -> p b c", p=P),
                )
                return ft

            for half in range(2):
                p1 = []
                for rc2 in range(2):
                    rc = half * 2 + rc2
                    acc = pacc.tile([C2, RCW], F32, name=f"p1_{rc}", tag="acc")
                    nc.tensor.matmul(
                        acc[:],
                        w2i0[:],
                        xm[:, rc * RCW:(rc + 1) * RCW],
                        start=True, stop=False,
                    )
                    p1.append(acc)

                kept = {}
                for ktb in range(NKB1):
                    if half == 1 and ktb < PF:
                        ft = pf_tiles[ktb]
                    elif half == 1 and ktb >= NKB1 - 4:
                        # pin the tiles pass 2 needs for its (oh=1, j=7)
                        # group so they are not re-read from HBM
                        ft = fpool.tile([P, KB1, HW_], BF16, name="ftk",
                                        tag="ftk", bufs=4)
                        nc.gpsimd.dma_start(
                            ft[:],
                            fltrT1_e[ktb * KB1 * P:(ktb + 1) * KB1 * P, :]
                            .rearrange("(b p) c -> p b c", p=P),
                        )
                        kept[ktb] = ft
                    else:
                        ft = ft_dma(half, ktb)
                    p1_conv(p1, ft, ktb)
                if half == 1:
                    kept_tiles = kept

                if half == 0:
                    # prefetch half-1's first tiles so the SDMA engines stay
                    # fed while the collective blocks the gpsimd queue
                    pf_tiles = [ft_dma(1, k) for k in range(PF)]
                else:
                    # prefetch pass-2's first tiles for the same reason
                    pf2_tiles = [ft2_dma(0, 0, 0, 0), ft2_dma(0, 0, 1, 0),
                                 ft2_dma(0, 1, 0, 0), ft2_dma(0, 1, 1, 0)]

                # epilogue: relu -> Y1 local (bf16) -> one all-gather
                y1h = ylpool.tile([P, 8, C2], BF16, name="y1h")
                for rc2 in range(2):
                    rc = half * 2 + rc2
                    xb1 = xbpool.tile([C2, RCW], F32, name="xb1")
                    nc.scalar.activation(
                        xb1[:], p1[rc2][:], mybir.ActivationFunctionType.Relu,
                        bias=bi0[:], scale=1.0,
                    )
                    for t in range(RC):  # node-subtiles of 128 in the chunk
                        psy = psmall.tile([P, C2], F32, name="psy", tag="psy")
                        nc.tensor.matmul(
                            psy[:],
                            xb1[:, t * P:(t + 1) * P],
                            w1i1[:],
                            start=True, stop=True,
                        )
                        nc.vector.tensor_copy(y1h[:, rc2 * RC + t, :], psy[:])
                gin = dram.tile([HW_, C2], BF16, name="gin", tag="gin", bufs=2)
                nc.sync.dma_start(
                    gin[:].rearrange("(t p) ch -> p t ch", p=P),
                    y1h[:],
                )
                gout = dram.tile(
                    [NCORES * HW_, C2], BF16, name="gout", tag="gout",
                    addr_space="Shared", bufs=2,
                )
                nc.gpsimd.collective_compute(
                    "AllGather", mybir.AluOpType.bypass,
                    replica_groups=RG,
                    ins=[gin[:].opt()], outs=[gout[:].opt()],
                )
                gouts.append(gout)

            outT = opool.tile([C, R], F32)

            # ---- pass 2: two output-column halves; the first half's
            # ---- epilogue hides under the second half's stream ----
            ygs = []
            for h in range(2):
                yg = ygpool.tile([P, NCORES * NB2G, C2], BF16, name="yg",
                                 bufs=2)
                nc.sync.dma_start(
                    yg[:], gouts[h][:].rearrange("(b p) ch -> p b ch", p=P)
                )
                ygs.append(yg)

            for oh in range(2):
                p2 = []
                for rc2 in range(2):
                    rc = oh * 2 + rc2
                    acc = pacc.tile([C2, RCW], F32, name=f"p2_{rc}", tag="acc")
                    nc.tensor.matmul(
                        acc[:],
                        w2i1[:],
                        xm[:, rc * RCW:(rc + 1) * RCW],
                        start=True, stop=False,
                    )
                    p2.append(acc)
                jorder = list(range(NCORES)) if oh == 0 \
                    else [NCORES - 1] + list(range(NCORES - 1))
                n_done = 0
                for h in range(2):
                    for j in jorder:
                        for q in range(2):
                            n_done += 1
                            if oh == 0 and h == 0 and j < 2:
                                ft = pf2_tiles[j * 2 + q]
                            elif oh == 1 and j == NCORES - 1:
                                # pinned from pass 1: rows j*2048+h*1024+q*512
                                # = ktb 28 + h*2 + q, cols [1024:2048]
                                ft = kept_tiles[28 + h * 2 + q]
                            else:
                                ft = ft2_dma(h, j, q, oh)
                            for t in range(4):
                                kt_in = q * 4 + t
                                last = (n_done == 2 * NCORES * 2) and (t == 3)
                                for rc2 in range(2):
                                    nc.tensor.matmul(
                                        p2[rc2][:],
                                        ygs[h][:, j * NB2G + kt_in, :],
                                        ft[:, t, rc2 * RCW:(rc2 + 1) * RCW],
                                        start=False, stop=last,
                                    )
                # epilogue for this output half
                for rc2 in range(2):
                    rc = oh * 2 + rc2
                    xb2 = xbpool.tile([C2, RCW], F32, name="xb2")
                    nc.scalar.activation(
                        xb2[:], p2[rc2][:], mybir.ActivationFunctionType.Relu,
                        bias=bi1h[:], scale=0.5,
                    )
                    # partition-shift stack-1 half to base 0 (DMA), then add
                    xs = xbpool.tile([C, RCW], F32, name="xs")
                    nc.sync.dma_start(xs[:], xb2[C:C2, :])
                    nc.vector.tensor_add(
                        outT[:, rc * RCW:(rc + 1) * RCW],
                        xb2[0:C, :], xs[:],
                    )
            nc.sync.dma_start(out_e[:], outT[:])

    nc.compile()
    return nc


def kernel(**inputs):
    x = np.ascontiguousarray(np.asarray(inputs["x"], dtype=np.float32))
    fltr = np.ascontiguousarray(np.asarray(inputs["fltr"], dtype=np.float32))

    def cat(a, b, axis=1):
        return np.ascontiguousarray(
            np.concatenate(
                [np.asarray(a, np.float32), np.asarray(b, np.float32)],
                axis=axis,
            )
        )

    w1i0 = cat(inputs["k0i0_w1"], inputs["k1i0_w1"])
    w1i1 = np.zeros((C2, C2), dtype=np.float32)
    w1i1[0:C, 0:C] = np.asarray(inputs["k0i1_w1"], np.float32)
    w1i1[C:C2, C:C2] = np.asarray(inputs["k1i1_w1"], np.float32)
    w2i0 = cat(inputs["k0i0_w2"], inputs["k1i0_w2"])
    w2i1 = cat(inputs["k0i1_w2"], inputs["k1i1_w2"])
    bi0 = cat(inputs["k0i0_b"], inputs["k1i0_b"], axis=0)[:, None]
    bi1h = 0.5 * cat(inputs["k0i1_b"], inputs["k1i1_b"], axis=0)[:, None]
    bi1h = np.ascontiguousarray(bi1h)
    xT = np.ascontiguousarray(x.T)

    if "nc" not in _CACHE:
        _CACHE["nc"] = _build()
    nc = _CACHE["nc"]

    in_maps = []
    for m in range(NCORES):
        rows = slice(m * R, (m + 1) * R)
        in_maps.append({
            "fltrt0": np.ascontiguousarray(fltr[m * R:m * R + R // 2, :].T),
            "fltrt1": np.ascontiguousarray(fltr[m * R + R // 2:(m + 1) * R, :].T),
            "xt": xT,
            "xtm": np.ascontiguousarray(x[rows, :].T),
            "w1i0": w1i0, "w1i1": w1i1, "w2i0": w2i0, "w2i1": w2i1,
            "bi0": bi0, "bi1h": bi1h,
        })

    import os
    import time
    trace = os.environ.get("ARMA_TRACE") == "1"
    last_exc = None
    for attempt in range(3):
        try:
            res = run_bass_kernel_spmd(
                nc, in_maps, core_ids=list(range(NCORES)), trace=trace,
            )
            break
        except Exception as e:  # transient NRT device errors: retry
            last_exc = e
            time.sleep(5.0)
    else:
        raise last_exc
    _CACHE["last_results"] = res
    out = np.concatenate(
        [np.asarray(res.results[m]["out"]).T for m in range(NCORES)], axis=0
    )
    return out

